# revision 1
# baseline (speedup 1.0000x reference)
"""MoE expert-pool kernel for Trainium2, 8 NeuronCores, expert-parallel.

Strategy:
  - Host: route tokens to experts (distinct (token,expert) pairs, combined
    routing weight per pair), gather per-expert token blocks, pad to a
    common capacity C, cast to bf16.
  - Device (per core = one expert): YT = W2^T @ gelu(W1^T @ XT + b1),
    all operands kept transposed so both weight matrices are used in their
    native layout as the stationary (lhsT) matmul operand. bf16 inputs,
    fp32 PSUM accumulation.
  - Host: scatter-add cw * (Y + b2) back to the [T, H] output.

Hardcoded problem shape: T=4096, H=1024, F=4096, E=8, K=2 (fp32 inputs).
"""

import sys
import types

import numpy as np
import ml_dtypes

H = 1024
F = 4096
E = 8
N_CORES = 8
PART = 128
TOK_CHUNK = 512  # fp32 PSUM bank = 512 columns


def _install_axon_trace_shim():
    """Make run_bass_kernel_spmd(trace=True) survive images that lack
    antenv.axon_hooks (tracing degrades gracefully if the hook .so is
    unavailable)."""
    try:
        import antenv.axon_hooks  # noqa: F401
        return
    except ImportError:
        pass
    mod = types.ModuleType("antenv.axon_hooks")
    mod._hook = None

    def set_axon_ntff_profile_hook(h):
        mod._hook = h

    def get_axon_ntff_profile_hook():
        return mod._hook

    mod.set_axon_ntff_profile_hook = set_axon_ntff_profile_hook
    mod.get_axon_ntff_profile_hook = get_axon_ntff_profile_hook
    sys.modules["antenv.axon_hooks"] = mod
    try:
        import antenv
        antenv.axon_hooks = mod
    except ImportError:
        pass
    try:
        from trn_agent_boot.trn_boot import _ntff_profile_via_ctypes
        mod._hook = _ntff_profile_via_ctypes("/opt/axon/libaxon_pjrt.so")
    except Exception:
        pass


_install_axon_trace_shim()

_PROGRAM_CACHE = {}


def _w1_groups():
    """W1 DMA column groups: a small first group (one m-tile) so the first
    matmul group is gated by minimal bytes, then 512-wide groups."""
    groups = [(0, PART), (PART, 512 - PART)]
    groups += [(g, 512) for g in range(512, F, 512)]
    return groups


def _w2_groups():
    return [(g, 512) for g in range(0, H, 512)]


def _pack_groups(w, kt, groups):
    """Pack a [kt*PART, cols] matrix into SBUF group-major layout
    [PART, kt*cols]: per group [p][(k, c)] contiguous."""
    w3 = w.reshape(kt, PART, w.shape[1])
    parts = [
        np.ascontiguousarray(
            w3[:, :, g0:g0 + gw].transpose(1, 0, 2).reshape(PART, kt * gw))
        for (g0, gw) in groups
    ]
    return np.ascontiguousarray(np.concatenate(parts, axis=1))


def _build_program(C):
    """Build + bacc-compile the per-core Bass program for capacity C."""
    import concourse.mybir as mybir
    import concourse.tile as tile
    from concourse import bacc

    bf16 = mybir.dt.bfloat16
    f32 = mybir.dt.float32

    KT1 = H // PART   # 8  k-tiles for mm1 (contract over H)
    MT1 = F // PART   # 32 m-tiles for mm1 (output partitions = F chunks)
    KT2 = F // PART   # 32 k-tiles for mm2 (contract over F)
    MT2 = H // PART   # 8  m-tiles for mm2 (output partitions = H chunks)

    # token chunks (PSUM free-dim limit 512 for fp32)
    chunks = []
    off = 0
    while off < C:
        n = min(TOK_CHUNK, C - off)
        chunks.append((off, n))
        off += n

    WARM_MMS = 66   # dummy matmuls to lift the HAM clock gate during DMA ramp

    nc = bacc.Bacc("TRN2", target_bir_lowering=False, debug=False,
                   num_devices=N_CORES)

    # All inputs are host-arranged group-major in SBUF layout ([p][k][cols]
    # per group, groups concatenated) so every DMA reads fully-contiguous
    # per-partition lines (128 big descriptors instead of 1024 small ones).
    w1_groups = _w1_groups()
    w2_groups = _w2_groups()
    xt_d = nc.dram_tensor("xt", [PART, KT1 * C], bf16, kind="ExternalInput")
    w1_d = nc.dram_tensor("w1", [PART, KT1 * F], bf16, kind="ExternalInput")
    w2_d = nc.dram_tensor("w2", [PART, KT2 * H], bf16, kind="ExternalInput")
    b1_d = nc.dram_tensor("b1t", [PART, MT1], f32, kind="ExternalInput")
    yt_d = nc.dram_tensor("yt", [H, C], f32, kind="ExternalOutput")

    with tile.TileContext(nc) as tc:
        with (
            tc.tile_pool(name="big", bufs=1) as big_pool,
            tc.tile_pool(name="consts", bufs=1) as consts,
            tc.tile_pool(name="stage", bufs=4) as stage_pool,
            tc.tile_pool(name="psum", bufs=4, space="PSUM") as psum_pool,
            tc.tile_pool(name="wpsum", bufs=1, space="PSUM") as wpsum_pool,
        ):
            gelu = mybir.ActivationFunctionType.Gelu

            # PE pre-warm: zero-tile matmuls keep the PE busy through the
            # HAM activity window so the real stream starts at 2.4 GHz.
            warm_sb = consts.tile([PART, PART], bf16)
            nc.vector.memset(warm_sb[:], 0.0)
            wps = wpsum_pool.tile([PART, PART], f32)
            for _ in range(WARM_MMS):
                nc.tensor.matmul(wps[:], warm_sb[:], warm_sb[:],
                                 start=True, stop=True)

            b1_sb = consts.tile([PART, MT1], f32)

            # SBUF tiles mirror the DRAM packed layout exactly, so every
            # DMA is 128 fully-contiguous runs (one per partition).
            xt_sb = big_pool.tile([PART, KT1 * C], bf16)
            w1_sb = big_pool.tile([PART, KT1 * F], bf16)
            w2_sb = big_pool.tile([PART, KT2 * H], bf16)
            h_sb = big_pool.tile([PART, MT1, TOK_CHUNK], bf16)

            # DMA order = consumption order. Critical prefix (gates the
            # first matmul group): chunk-0 tokens + W1's first m-tile.
            t00, tn0 = chunks[0]
            nc.sync.dma_start(xt_sb[:, t00 * KT1:(t00 + tn0) * KT1],
                              xt_d.ap()[:, t00 * KT1:(t00 + tn0) * KT1])
            for gi, (g0, gw) in enumerate(w1_groups):
                eng = nc.gpsimd if gi == 0 else nc.sync
                eng.dma_start(w1_sb[:, g0 * KT1:(g0 + gw) * KT1],
                              w1_d.ap()[:, g0 * KT1:(g0 + gw) * KT1])
                if gi == 1:
                    nc.gpsimd.dma_start(b1_sb[:], b1_d.ap())
            for (t0, tn) in chunks[1:]:
                nc.sync.dma_start(xt_sb[:, t0 * KT1:(t0 + tn) * KT1],
                                  xt_d.ap()[:, t0 * KT1:(t0 + tn) * KT1])
            for (g0, gw) in w2_groups:
                nc.sync.dma_start(w2_sb[:, g0 * KT2:(g0 + gw) * KT2],
                                  w2_d.ap()[:, g0 * KT2:(g0 + gw) * KT2])

            def xt_slice(t0, tn, k):
                # tokens [t0, t0+tn) of k-slab k (chunk-major packing)
                base = t0 * KT1 + k * tn
                return xt_sb[:, base:base + tn]

            def w_slice(w_sb, groups, kt, m, k):
                # m-tile m, k-slab k from group-major packing
                for (g0, gw) in groups:
                    if g0 <= m * PART < g0 + gw:
                        base = g0 * kt + k * gw + (m * PART - g0)
                        return w_sb[:, base:base + PART]
                raise AssertionError

            for (t0, tn) in chunks:
                # mm1 + gelu: h = gelu(W1^T X + b1) for this token chunk
                for m in range(MT1):
                    ps = psum_pool.tile([PART, TOK_CHUNK], f32, tag="ps",
                                        name="ps")
                    for k in range(KT1):
                        nc.tensor.matmul(
                            ps[:, :tn],
                            w_slice(w1_sb, w1_groups, KT1, m, k),
                            xt_slice(t0, tn, k),
                            start=(k == 0), stop=(k == KT1 - 1))
                    nc.scalar.activation(
                        h_sb[:, m, :tn], ps[:, :tn], gelu,
                        bias=b1_sb[:, m:m + 1], scale=1.0)

                # mm2: yt = W2^T h for this token chunk
                for m in range(MT2):
                    ps = psum_pool.tile([PART, TOK_CHUNK], f32, tag="ps",
                                        name="ps")
                    for k in range(KT2):
                        nc.tensor.matmul(
                            ps[:, :tn],
                            w_slice(w2_sb, w2_groups, KT2, m, k),
                            h_sb[:, k, :tn],
                            start=(k == 0), stop=(k == KT2 - 1))
                    out_sb = stage_pool.tile([PART, TOK_CHUNK], f32,
                                             tag="out", name="out")
                    last = (m == MT2 - 1) and (t0 + tn >= C)
                    if last:
                        # tail-critical: copy+DMA in halves so the first
                        # DMA overlaps the second copy
                        h0 = tn // 2
                        for (a, b) in ((0, h0), (h0, tn)):
                            nc.vector.tensor_copy(out_sb[:, a:b], ps[:, a:b])
                            nc.sync.dma_start(
                                yt_d.ap()[m * PART:(m + 1) * PART,
                                          t0 + a:t0 + b],
                                out_sb[:, a:b])
                    else:
                        nc.vector.tensor_copy(out_sb[:, :tn], ps[:, :tn])
                        nc.sync.dma_start(
                            yt_d.ap()[m * PART:(m + 1) * PART, t0:t0 + tn],
                            out_sb[:, :tn])

    nc.compile()
    return nc


def _route(expert_weights, selected_experts):
    """Distinct (token, expert) pairs with combined weights.

    Returns per-expert (token_ids, combined_weights)."""
    se = np.asarray(selected_experts).astype(np.int64)
    ew = np.asarray(expert_weights).astype(np.float32)
    routes = []
    for e in range(E):
        hit = (se == e)  # [T, K]
        tok = np.nonzero(hit.any(axis=1))[0]
        cw = (ew * hit).sum(axis=1)[tok]
        routes.append((tok, cw))
    return routes


def kernel(hidden_states, expert_weights, W1, b1, W2, b2, selected_experts):
    from concourse.bass_utils import run_bass_kernel_spmd

    hs = np.asarray(hidden_states)
    out_dtype = hs.dtype
    hs = hs.astype(np.float32)
    W1 = np.asarray(W1).astype(np.float32)
    b1 = np.asarray(b1).astype(np.float32)
    W2 = np.asarray(W2).astype(np.float32)
    b2 = np.asarray(b2).astype(np.float32)

    T = hs.shape[0]
    assert hs.shape[1] == H and W1.shape == (E, H, F) and W2.shape == (E, F, H)

    routes = _route(expert_weights, selected_experts)
    max_n = max(len(tok) for tok, _ in routes)
    C = max(PART, max_n)

    if C not in _PROGRAM_CACHE:
        _PROGRAM_CACHE[C] = _build_program(C)
    nc = _PROGRAM_CACHE[C]

    bf16 = ml_dtypes.bfloat16
    KT1 = H // PART

    # token chunks must mirror _build_program's chunking
    chunk_sizes = []
    off = 0
    while off < C:
        n = min(TOK_CHUNK, C - off)
        chunk_sizes.append(n)
        off += n

    in_maps = []
    for e in range(E):
        tok, _ = routes[e]
        xt = np.zeros((H, C), dtype=bf16)
        if len(tok):
            xt[:, :len(tok)] = hs[tok].T.astype(bf16)
        # chunk-major SBUF layout: per chunk [p][k][tok], concatenated
        xt_groups = []
        t0 = 0
        for tn in chunk_sizes:
            xt_groups.append((t0, tn))
            t0 += tn
        xt_host = _pack_groups(xt, KT1, xt_groups)

        in_maps.append({
            "xt": xt_host,
            "w1": _pack_groups(W1[e].astype(bf16), KT1, _w1_groups()),
            "w2": _pack_groups(W2[e].astype(bf16), F // PART, _w2_groups()),
            "b1t": np.ascontiguousarray(b1[e].reshape(F // PART, PART).T),
        })

    res = run_bass_kernel_spmd(nc, in_maps, core_ids=list(range(N_CORES)))

    out = np.zeros((T, H), dtype=np.float32)
    for e in range(E):
        tok, cw = routes[e]
        if len(tok) == 0:
            continue
        yt = res.results[e]["yt"][:, :len(tok)].astype(np.float32)
        out[tok] += cw[:, None] * (yt.T + b2[e][None, :])
    return out.astype(out_dtype)

